# revision 5
# baseline (speedup 1.0000x reference)
"""Trainium2 Bass kernel for DigitCapsuleLayer (single routing iteration).

Math: with num_iterations == 1 the routing coefficients are uniform 1/R, so

    v[b,c,o] = squash( (1/R) * sum_{r,i} x[b,r,i] * W[0,r,c,o,i] )

i.e. one big [B=128, K=32768] x [K=32768, N=1024] fp32 matmul followed by a
tiny squash nonlinearity.  W is 128 MB and read exactly once -> the kernel is
HBM-bound at ~144 MB of total traffic.

Sharding (8 cores): split the contraction dim K = (routes x incap) so each
core reads a distinct 16 MB slice of W (and a 2 MB slice of x), computes a
[128, 1024] partial product, then an on-device ReduceScatter over the 8 cores
sums the partials and leaves each core with a [16, 1024] batch-slice of the
reduced sum.  Each core applies the squash on its slice and the host just
concatenates the 8 output slices (pure data movement).
"""

import numpy as np

import concourse.bacc as bacc
import concourse.bass as bass
import concourse.bass_utils as bass_utils
import concourse.mybir as mybir
import concourse.tile as tile

# Problem shape (hardcoded per the kernel contract).
B, R, C, I, O = 128, 2048, 32, 16, 32
NCORES = 8
RSH = R // NCORES          # 256 routes per core
KS = RSH * I               # 4096 contraction rows per core
KC = KS // 128             # 32 k-chunks of 128
N = C * O                  # 1024
BS = B // NCORES           # 16 batch rows per core after ReduceScatter

# PE fp32 runs at 4 cycles/row; float32r streams at 1 cycle/row for N>=256
# with ~2^-16-level relative error (fp32 data split in HW).  The contraction
# accumulates in fp32 PSUM either way.
USE_F32R = True
W_DMA_CHUNKS = 8           # number of DMA transfers for the 16 MB W slice


def _build_program():
    nc = bacc.Bacc(
        "TRN2", target_bir_lowering=False, debug=False, num_devices=NCORES
    )
    f32 = mybir.dt.float32
    mm_dt = mybir.dt.float32r if USE_F32R else mybir.dt.float32

    xT = nc.dram_tensor("xT", [128, KC * B], mm_dt, kind="ExternalInput").ap()
    Wt = nc.dram_tensor("Wt", [128, KC, N], mm_dt, kind="ExternalInput").ap()
    out = nc.dram_tensor("out", [BS, N], f32, kind="ExternalOutput").ap()

    with tile.TileContext(nc) as tc:
        with (
            tc.tile_pool(name="xpool", bufs=1) as xpool,
            tc.tile_pool(name="wpool", bufs=1) as wpool,
            tc.tile_pool(name="spool", bufs=1) as spool,
            tc.tile_pool(name="qpool", bufs=1) as qpool,
            tc.tile_pool(name="psum", bufs=1, space="PSUM") as psum_pool,
            tc.tile_pool(name="dram", bufs=1, space="DRAM") as dram_pool,
        ):
            # x slice resident in SBUF: [p=k%128, (kc, b)] = 2 MB.
            x_sb = xpool.tile([128, KC * B], mm_dt)
            nc.sync.dma_start(x_sb[:], xT[:])

            # W slice resident in SBUF: [p, kc, (c,o)] = 16 MB, streamed in
            # W_DMA_CHUNKS transfers so matmuls start early.
            w_sb = wpool.tile([128, KC, N], mm_dt)
            step = KC // W_DMA_CHUNKS
            for g in range(W_DMA_CHUNKS):
                nc.sync.dma_start(
                    w_sb[:, g * step : (g + 1) * step, :],
                    Wt[:, g * step : (g + 1) * step, :],
                )

            ps0 = psum_pool.tile([128, 512], f32)
            ps1 = psum_pool.tile([128, 512], f32)
            for kc in range(KC):
                lhsT = x_sb[:, kc * B : (kc + 1) * B]
                rhs0 = w_sb[:, kc, 0:512]
                rhs1 = w_sb[:, kc, 512:1024]
                nc.tensor.matmul(
                    ps0, lhsT, rhs0, start=(kc == 0), stop=(kc == KC - 1)
                )
                nc.tensor.matmul(
                    ps1, lhsT, rhs1, start=(kc == 0), stop=(kc == KC - 1)
                )

            # Scale partial sums by 1/R while copying PSUM -> SBUF.
            s_sb = spool.tile([128, N], f32)
            nc.scalar.mul(s_sb[:, 0:512], ps0[:], 1.0 / R)
            nc.scalar.mul(s_sb[:, 512:1024], ps1[:], 1.0 / R)

            # Sum partials across cores; rank m keeps batch rows
            # [m*16, (m+1)*16).
            cc_in = dram_pool.tile([B, N], f32)
            cc_out = dram_pool.tile([BS, N], f32)
            nc.sync.dma_start(cc_in[:], s_sb[:])
            nc.gpsimd.collective_compute(
                "ReduceScatter",
                mybir.AluOpType.add,
                replica_groups=[list(range(NCORES))],
                ins=[cc_in.opt()],
                outs=[cc_out.opt()],
            )

            # Squash over the out-capsule dim on the [16, 1024] slice.
            # Spread (b, c) over 128 partitions: p = (b, c_hi) with c_hi = c//4.
            sv = qpool.tile([128, 4, 32], f32)
            nc.sync.dma_start(
                sv[:], cc_out.rearrange("b (ch cl o) -> (b ch) cl o", ch=8, cl=4, o=32)
            )
            s2 = qpool.tile([128, 4, 32], f32)
            nc.vector.tensor_mul(out=s2[:], in0=sv[:], in1=sv[:])
            sq = qpool.tile([128, 4], f32)
            nc.vector.reduce_sum(sq[:], s2[:], axis=mybir.AxisListType.X)
            rt = qpool.tile([128, 4], f32)
            nc.scalar.sqrt(rt[:], sq[:])
            den = qpool.tile([128, 4], f32)
            nc.scalar.add(den[:], sq[:], 1.0)
            rec = qpool.tile([128, 4], f32)
            nc.vector.reciprocal(rec[:], den[:])
            fac = qpool.tile([128, 4], f32)
            nc.vector.tensor_mul(out=fac[:], in0=rt[:], in1=rec[:])
            v = qpool.tile([128, 4, 32], f32)
            nc.vector.tensor_tensor(
                v[:],
                sv[:],
                fac[:, :, None].to_broadcast((128, 4, 32)),
                mybir.AluOpType.mult,
            )
            nc.sync.dma_start(
                out.rearrange("b (ch cl o) -> (b ch) cl o", ch=8, cl=4, o=32), v[:]
            )

    nc.compile()
    return nc


def _shard_inputs(x: np.ndarray, W: np.ndarray):
    """Per-core input layouts (pure data movement on host).

    Contraction index within core m: k = kc*128 + p with p = (rp, i),
    rp in [0,8); global route r = m*256 + kc*8 + rp.
    """
    in_maps = []
    for m in range(NCORES):
        xm = x[:, m * RSH : (m + 1) * RSH, :]          # (b, rr, i)
        xm = xm.reshape(B, KC, 8, I)                   # (b, kc, rp, i)
        x_prep = np.ascontiguousarray(
            xm.transpose(2, 3, 1, 0)                   # (rp, i, kc, b)
        ).reshape(128, KC * B)

        Wm = W[0, m * RSH : (m + 1) * RSH]             # (rr, c, o, i)
        Wm = Wm.reshape(KC, 8, C, O, I)                # (kc, rp, c, o, i)
        w_prep = np.ascontiguousarray(
            Wm.transpose(1, 4, 0, 2, 3)                # (rp, i, kc, c, o)
        ).reshape(128, KC, N)

        in_maps.append({"xT": x_prep, "Wt": w_prep})
    return in_maps


_CACHED_NC = None


def _get_nc():
    global _CACHED_NC
    if _CACHED_NC is None:
        _CACHED_NC = _build_program()
    return _CACHED_NC


def kernel(x: np.ndarray, W: np.ndarray, _trace: bool = False):
    x = np.ascontiguousarray(np.asarray(x, dtype=np.float32))
    W = np.ascontiguousarray(np.asarray(W, dtype=np.float32))
    nc = _get_nc()
    in_maps = _shard_inputs(x, W)
    res = bass_utils.run_bass_kernel_spmd(
        nc, in_maps, core_ids=list(range(NCORES)), trace=_trace
    )
    full = np.concatenate([res.results[m]["out"] for m in range(NCORES)], axis=0)
    out = full.reshape(B, C, O, 1)
    if _trace:
        return out, res
    return out
